# revision 1
# baseline (speedup 1.0000x reference)
"""Distributed Trainium2 Bass kernel for nn_ClosedFlyLoop.

Strategy (8 NeuronCores, shard X into 8 blocks of 256):
 - host: symmetrize v, split y into (m[4], s), pad X edge-replicate by H=25,
   cut per-core overlapping slabs [7, 1024, 306] (halo covers smooth+diff+
   post-smooth supports; the AP_CUT zeroing erases the only global-edge
   discrepancy), convert to bf16.
 - device, per core (no collectives needed):
     stage 1a: conv along y (circular) as banded matmuls that also transpose
               layout [y,x] -> [x,y']  (lhsT = data chunk, rhs = Toeplitz band)
     stage 1b: conv along x as banded matmuls transposing back [x,y]->[y,x'],
               producing the 14 gradient fields (sign/scale folds baked into
               the band matrices)
     algebra:  pointwise ClosedFlyLoop RHS on VectorE/ScalarE/GpSimd (bf16)
     stage 4a/4b: final Gaussian smooth of the 5 masked pre-fields (same two
               banded-matmul tricks), mask folded into the 4a PSUM evac as a
               per-partition scalar multiply
 - host: concatenate per-core [5, 1024, 256] f32 outputs along X.
"""
import numpy as np
import ml_dtypes

import concourse.bass as bass
import concourse.bacc as bacc
import concourse.mybir as mybir
from concourse import tile
from concourse.bass_utils import run_bass_kernel_spmd

BF16 = ml_dtypes.bfloat16
F32 = np.float32

Y, X = 1024, 2048
NCORES = 8
XS = X // NCORES            # 256
RAD = 12                    # gauss radius: int(4.0*3.0+0.5)
H = 2 * RAD + 1             # 25
W_IN = XS + 2 * H           # 306
W_ALG = XS + 2 * RAD        # 280
OFF1B = H - RAD             # 13: slab-coord offset of alg window
AP_CUT = 15
YT = Y // 128               # 8 y tiles
XT_IN = [(0, 128), (128, 128), (256, W_IN - 256)]       # x tiles of slab (128,128,50)
XT_ALG = [(0, 128), (128, 128), (256, W_ALG - 256)]     # x tiles of alg width (128,128,24)
HALF = 512                  # y' half width for stage a psum


def _gauss():
    r = RAD
    x = np.arange(-r, r + 1, dtype=np.float64)
    k = np.exp(-0.5 * (x / 3.0) ** 2)
    k = (k / k.sum()).astype(np.float64)
    dk = np.convolve(k, [-0.5, 0.0, 0.5])
    return k.astype(F32), dk.astype(F32)


KERN, DKERN = _gauss()      # 25 taps (rad 12), 27 taps (rad 13)


# ---------------- band submatrix machinery (host) ----------------
class BandPack:
    """Dedup + pack all band submatrices into one [128, K] bf16 constant."""

    def __init__(self):
        self.blocks = {}
        self.cols = []
        self.total = 0

    def add(self, sub):
        sub16 = np.ascontiguousarray(sub.astype(BF16))
        key = (sub16.shape, sub16.tobytes())
        if key not in self.blocks:
            pad = np.zeros((128, sub16.shape[1]), dtype=BF16)
            pad[: sub16.shape[0]] = sub16
            self.blocks[key] = self.total
            self.cols.append(pad)
            self.total += sub16.shape[1]
        return self.blocks[key]

    def packed(self):
        return np.concatenate(self.cols, axis=1)


def band_subs_y(pack, ker, scale):
    """Circular conv along y (1024). Returns per half h: list of
    (chunk k, col a, col b, packed offset, rows) for rhs = B[krows, h*512+a : h*512+b]."""
    r = ker.shape[0] // 2
    B = np.zeros((Y, Y), dtype=F32)
    for j in range(Y):
        for t in range(-r, r + 1):
            B[(j + t) % Y, j] = ker[r + t] * scale
    out = []
    for h in range(2):
        subs = []
        for k in range(YT):
            sub = B[k * 128:(k + 1) * 128, h * HALF:(h + 1) * HALF]
            cols = np.flatnonzero(np.any(sub != 0.0, axis=0))
            if cols.size == 0:
                continue
            a, b = int(cols[0]), int(cols[-1] + 1)
            assert b - a == cols.size
            off = pack.add(sub[:, a:b])
            subs.append((k, a, b, off, 128))
        out.append(subs)
    return out


def band_subs_x(pack, ker, scale, n_in, n_out, off_in):
    """conv along x: out[j] = sum_t kc[t] in[j + off_in + t].
    Returns list of (chunk k, col a, col b, packed offset, rows)."""
    r = ker.shape[0] // 2
    B = np.zeros((n_in, n_out), dtype=F32)
    for j in range(n_out):
        for t in range(-r, r + 1):
            i = j + off_in + t
            if 0 <= i < n_in:
                B[i, j] = ker[r + t] * scale
    subs = []
    nchunks = (n_in + 127) // 128
    for k in range(nchunks):
        rows = min(128, n_in - k * 128)
        sub = B[k * 128:k * 128 + rows, :]
        cols = np.flatnonzero(np.any(sub != 0.0, axis=0))
        if cols.size == 0:
            continue
        a, b = int(cols[0]), int(cols[-1] + 1)
        assert b - a == cols.size
        off = pack.add(sub[:, a:b])
        subs.append((k, a, b, off, rows))
    return subs


# channel order in slab: m00 m01 m10 m11 s v0 v1
# stage-1a D-variant scale per channel (folds signs), stage-1b DK scale per channel
CH_DY_SCALE = [-1.0, -1.0, -1.0, -1.0, -1.0, 1.0, 0.5]   # conv_y(DK) scale
CH_DX_SCALE = [-1.0, -1.0, -1.0, -1.0, -1.0, -0.5, 1.0]  # conv_x(DK) scale


def build_graph():
    pack = BandPack()
    sub_ky = band_subs_y(pack, KERN, 1.0)                 # shared smooth-y (also stage 4a)
    # dy via shifted-view matmuls: fold +-0.5*CH_DY_SCALE into the 1b Kx band
    sub_kxd = {}
    for sc in sorted(set(0.5 * s for s in CH_DY_SCALE) | set(-0.5 * s for s in CH_DY_SCALE)):
        sub_kxd[sc] = band_subs_x(pack, KERN, sc, W_IN, W_ALG, OFF1B)
    sub_dkx = {}
    for sc in sorted(set(CH_DX_SCALE)):
        sub_dkx[sc] = band_subs_x(pack, DKERN, sc, W_IN, W_ALG, OFF1B)
    sub_kx4 = band_subs_x(pack, KERN, 1.0, W_ALG, XS, RAD)       # stage 4b
    bands_np = pack.packed()
    KTOT = bands_np.shape[1]

    nc = bacc.Bacc()
    x_ext = nc.declare_dram_parameter("x", [7, Y, W_IN], mybir.dt.bfloat16, isOutput=False)
    bands_ext = nc.declare_dram_parameter("bands", [128, KTOT], mybir.dt.bfloat16, isOutput=False)
    mask_ext = nc.declare_dram_parameter("mask", [128, 3], mybir.dt.float32, isOutput=False)
    out_ext = nc.declare_dram_parameter("out", [5, Y, XS], mybir.dt.bfloat16, isOutput=True)

    bf = mybir.dt.bfloat16
    f32 = mybir.dt.float32
    TT = mybir.AluOpType
    evac_ctr = [0]

    with tile.TileContext(nc) as tc:
        with (
            tc.tile_pool(name="const", bufs=1) as constp,
            tc.tile_pool(name="slab", bufs=1) as slabp,
            tc.tile_pool(name="gyt", bufs=1) as gytp,
            tc.tile_pool(name="alg", bufs=3) as algp,
            tc.tile_pool(name="pre", bufs=1) as prep,
            tc.tile_pool(name="gyt2", bufs=3) as gyt2p,
            tc.tile_pool(name="outs", bufs=2) as outsp,
            tc.tile_pool(name="ps", bufs=4, space=bass.MemorySpace.PSUM) as psp,
        ):
            bands = constp.tile([128, KTOT], bf, tag="bands", name="bands")
            nc.sync.dma_start(bands[:, :], bands_ext[:, :])
            # PE p-state warm-up: ~3us of dummy matmuls on the bands constant
            # while the slab DMAs stream, so the real 1a convs start at full
            # clock. Result is never read.
            wps = psp.tile([128, HALF], f32, tag="ps", name="ps")
            nc.tensor.matmul(wps[:, :], bands[:128, 0:128], bands[:128, 0:HALF],
                             start=True, stop=True)
            maskt = constp.tile([128, 3], f32, tag="mask", name="mask")
            nc.sync.dma_start(maskt[:, :], mask_ext[:, :])

            # persistent slab: one wide tile per channel [128, YT*W_IN]; one DMA
            # per channel (128 descriptors of 8*612B rows vs 8 DMAs of 625ns
            # HWDGE overhead each).
            slabw = [slabp.tile([128, YT * W_IN], bf, tag=f"slabw{c}", name=f"slabw{c}")
                     for c in range(7)]
            for c in range(7):
                nc.sync.dma_start(
                    slabw[c][:, :],
                    x_ext[c].rearrange("(t p) x -> p t x", p=128))
            slab = [[slabw[c][:, t * W_IN:(t + 1) * W_IN] for t in range(YT)]
                    for c in range(7)]

            evac_mode = ["head"]

            def evac(dst_ap, src_ap, scale_ap=None):
                # engine deterministic per psum slot (bufs=4 rotation) so each
                # slot is always released by the same engine -> 1 WAR wait.
                slot = evac_ctr[0] % 4
                evac_ctr[0] += 1
                if (slot < 2) if evac_mode[0] == "tail" else (slot < 2):
                    if scale_ap is None:
                        nc.scalar.copy(dst_ap, src_ap)
                    else:
                        nc.scalar.activation(dst_ap, src_ap,
                                             mybir.ActivationFunctionType.Copy,
                                             scale=scale_ap)
                else:
                    if scale_ap is None:
                        nc.vector.tensor_copy(dst_ap, src_ap)
                    else:
                        nc.vector.tensor_scalar(dst_ap, src_ap, scale_ap, None, TT.mult)

            def conv_group(psum_ap, subs, lhsT_fn):
                n = len(subs)
                for i, (k, a, b, off, rows) in enumerate(subs):
                    nc.tensor.matmul(
                        psum_ap[:, a:b],
                        lhsT_fn(k, rows),
                        bands[:rows, off:off + b - a],
                        start=(i == 0),
                        stop=(i == n - 1),
                    )

            # ---------------- stage 1a: conv_y (Ky only) for all channels ----------------
            # dy rides a raw circular 3-tap diff of gyk along the free y dim;
            # the 0.5 and per-channel signs are folded into the 1b Kx bands.
            # gyk padded: col p holds y = p-1, cols 0/1025 are the circular
            # wrap copies. The dy 3-tap diff is NOT materialized: the 1b
            # dy-path runs two column-shifted matmul groups with +-0.5-scaled
            # bands instead (the diff commutes with the x-conv).
            gyk_all = []
            for c in range(7):
                gyk = [gytp.tile([128, 1026], bf, tag=f"gyk{c}_{xt}", name=f"gyk{c}_{xt}") for xt in range(3)]
                gyk_all.append(gyk)
                for xt, (x0, xw) in enumerate(XT_IN):
                    for h in range(2):
                        ps = psp.tile([128, HALF], f32, tag="ps", name="ps")
                        conv_group(
                            ps[:xw, :], sub_ky[h],
                            lambda k, rows: slab[c][k][:, x0:x0 + xw])
                        evac(gyk[xt][:xw, 1 + h * HALF:1 + (h + 1) * HALF], ps[:xw, :])
                    g = gyk[xt]
                    nc.gpsimd.tensor_copy(g[:xw, 0:1], g[:xw, 1024:1025])
                    nc.gpsimd.tensor_copy(g[:xw, 1025:1026], g[:xw, 1:2])

            # ------- stage 1b + algebra, y-tile-PAIR major: the pointwise
            # algebra runs on [128, 2*280] two-tile batches, halving the
            # per-op fixed cost and the instruction count. PSUM pairs and
            # their evacuations stay per-tile.
            pre = [[prep.tile([128, 2 * W_ALG], bf, tag=f"pre{f}_{tp}", name=f"pre{f}_{tp}")
                    for tp in range(YT // 2)] for f in range(5)]
            A = slice(OFF1B, OFF1B + W_ALG)
            dxsubs = {c: sub_dkx[CH_DX_SCALE[c]] for c in range(7)}

            def _emit(ps_view, t, specs):
                first = True
                for gi, (gt, subs, coff) in enumerate(specs):
                    n = len(subs)
                    for i, (k, a, b, off, rows) in enumerate(subs):
                        nc.tensor.matmul(
                            ps_view[:, a:b],
                            gt[k][:rows, t * 128 + coff:t * 128 + coff + 128],
                            bands[:rows, off:off + b - a],
                            start=first,
                            stop=(gi == len(specs) - 1 and i == n - 1),
                        )
                        first = False

            def dy_specs(c):
                sc = 0.5 * CH_DY_SCALE[c]
                return [(gyk_all[c], sub_kxd[sc], 2), (gyk_all[c], sub_kxd[-sc], 0)]

            def grad_pair(t, specsA, specsB):
                """Two conv groups into one 2-bank psum tile (separate zero
                regions at f32 cols 0 and 512) -> one strided evac possible."""
                ps = psp.tile([128, 2 * HALF], f32, tag="pg", name="pg", bufs=2)
                _emit(ps[:, 0:HALF], t, specsA)
                _emit(ps[:, HALF:2 * HALF], t, specsB)
                return ps

            def pair_evac(ps, dst):
                """One ACT copy of both banks' [:, :W_ALG] into dst [128, 2*W_ALG]."""
                srcv = ps.rearrange("p (b x) -> p b x", b=2)[:, :, 0:W_ALG]
                dstv = dst.rearrange("p (b x) -> p b x", x=W_ALG)
                nc.scalar.copy(dstv, srcv)

            for tp in range(YT // 2):
                ta, tb = 2 * tp, 2 * tp + 1

                def sv2(c):
                    return slabw[c].rearrange("p (t x) -> p t x", x=W_IN)[
                        :, ta:tb + 1, OFF1B:OFF1B + W_ALG]

                def tmp(tag):
                    return algp.tile([128, 2 * W_ALG], bf, tag=tag, name=tag)

                def v3(t_):
                    return t_.rearrange("p (t x) -> p t x", x=W_ALG)

                ACT_C = mybir.ActivationFunctionType.Copy
                # ACT affines (no psum deps), batched over the tile pair
                c1 = tmp("c1"); c2 = tmp("c2"); c3 = tmp("c3"); c4 = tmp("c4")
                nc.scalar.activation(v3(c1), sv2(4), ACT_C, bias=-0.11, scale=0.099)
                nc.scalar.activation(v3(c2), sv2(4), ACT_C, bias=0.767, scale=0.055)
                nc.scalar.activation(v3(c3), sv2(4), ACT_C, bias=0.732, scale=-0.59)
                nc.scalar.activation(v3(c4), sv2(4), ACT_C, bias=0.069, scale=-0.048)
                # Pool: raw combinations, batched
                trm = tmp("trm"); u1 = tmp("u1"); u2 = tmp("u2")
                nc.gpsimd.tensor_tensor(v3(trm), sv2(0), sv2(3), TT.add)
                nc.gpsimd.tensor_tensor(v3(u1), sv2(1), sv2(2), TT.add)
                nc.gpsimd.tensor_tensor(v3(u2), sv2(3), sv2(0), TT.subtract)

                # w and trE pairs for both tiles; per-tile evac/psum-read into
                # paired tiles, then batched combinations
                wf2 = tmp("wf2"); t1 = tmp("t1")
                for ti, t in enumerate((ta, tb)):
                    ps_wt = grad_pair(
                        t,
                        [(gyk_all[5], dxsubs[5], 1)] + dy_specs(6),
                        dy_specs(5) + [(gyk_all[6], dxsubs[6], 1)])
                    nc.scalar.copy(wf2[:, ti * W_ALG:(ti + 1) * W_ALG], ps_wt[:, :W_ALG])
                    nc.vector.tensor_tensor(
                        t1[:, ti * W_ALG:(ti + 1) * W_ALG],
                        ps_wt[:, HALF:HALF + W_ALG], c2[:, ti * W_ALG:(ti + 1) * W_ALG],
                        TT.mult)
                wu1 = tmp("wu1"); wu2 = tmp("wu2")
                nc.vector.tensor_tensor(wu1[:, :], wf2[:, :], u1[:, :], TT.mult)
                nc.vector.tensor_tensor(wu2[:, :], wf2[:, :], u2[:, :], TT.mult)
                Ac = tmp("Ac")
                nc.gpsimd.tensor_tensor(c3[:, :], c3[:, :], trm[:, :], TT.mult)
                nc.vector.tensor_tensor(t1[:, :], t1[:, :], c1[:, :], TT.add)
                nc.vector.tensor_tensor(Ac[:, :], t1[:, :], c3[:, :], TT.add)
                nc.gpsimd.tensor_tensor(c4[:, :], c4[:, :], trm[:, :], TT.mult)
                Cc = c4

                # sdot: per-tile pair evacs into gs2, batched products
                gs2 = algp.tile([128, 4 * W_ALG], bf, tag="gs2", name="gs2")
                gs2v = gs2.rearrange("p (t b x) -> p t b x", b=2, x=W_ALG)
                for ti, t in enumerate((ta, tb)):
                    ps_s = grad_pair(t, dy_specs(4),
                                     [(gyk_all[4], dxsubs[4], 1)])
                    srcv = ps_s.rearrange("p (b x) -> p b x", b=2)[:, :, 0:W_ALG]
                    nc.scalar.copy(gs2v[:, ti, :, :], srcv)
                sd1 = tmp("q1"); sd2 = tmp("q2")
                nc.vector.tensor_tensor(v3(sd1), gs2v[:, :, 0, :], sv2(5), TT.mult)
                nc.vector.tensor_tensor(v3(sd2), gs2v[:, :, 1, :], sv2(6), TT.mult)
                nc.vector.tensor_tensor(pre[4][tp][:, :], sd1[:, :], sd2[:, :], TT.add)

                for ch in range(4):
                    gab2 = algp.tile([128, 4 * W_ALG], bf, tag="gab2", name="gab2", bufs=4)
                    gab2v = gab2.rearrange("p (t b x) -> p t b x", b=2, x=W_ALG)
                    for ti, t in enumerate((ta, tb)):
                        ps_ab = grad_pair(t, dy_specs(ch),
                                          [(gyk_all[ch], dxsubs[ch], 1)])
                        srcv = ps_ab.rearrange("p (b x) -> p b x", b=2)[:, :, 0:W_ALG]
                        nc.scalar.copy(gab2v[:, ti, :, :], srcv)
                    q1 = tmp("q1"); q2 = tmp("q2"); r = tmp("r")
                    nc.vector.tensor_tensor(v3(q1), gab2v[:, :, 0, :], sv2(5), TT.mult)
                    nc.vector.tensor_tensor(v3(q2), gab2v[:, :, 1, :], sv2(6), TT.mult)
                    nc.gpsimd.tensor_tensor(v3(r), v3(Ac), sv2(ch), TT.mult)
                    nc.vector.tensor_tensor(q1[:, :], q1[:, :], q2[:, :], TT.add)
                    nc.vector.tensor_tensor(q1[:, :], q1[:, :], r[:, :], TT.add)
                    p = pre[ch][tp]
                    if ch == 0:
                        nc.vector.tensor_tensor(q1[:, :], q1[:, :], wu1[:, :], TT.subtract)
                        nc.vector.tensor_tensor(p[:, :], q1[:, :], Cc[:, :], TT.add)
                    elif ch == 3:
                        nc.vector.tensor_tensor(p[:, :], q1[:, :], wu1[:, :], TT.add)
                    else:
                        nc.vector.tensor_tensor(p[:, :], q1[:, :], wu2[:, :], TT.subtract)

            # ---------------- stage 4: final smooth of 5 fields ----------------
            # 4a: 2-bank psum tiles from the now-idle pg pool, one fused
            # mask-scale evac per (f, xt) alternating ACT/DVE.
            # 4b: two y-tiles packed per single psum bank, one evac per pair.
            n4 = [0]
            for f in range(5):
                gy2 = [gyt2p.tile([128, 1024], bf, tag=f"gy2{xt}", name=f"gy2{xt}") for xt in range(3)]
                for xt, (x0, xw) in enumerate(XT_ALG):
                    ps = psp.tile([128, 2 * HALF], f32, tag="pg", name="pg", bufs=2)
                    for h in range(2):
                        conv_group(
                            ps[:xw, h * HALF:(h + 1) * HALF], sub_ky[h],
                            lambda k, rows: pre[f][k // 2][:, (k % 2) * W_ALG + x0:(k % 2) * W_ALG + x0 + xw])
                    sc = maskt[:xw, xt:xt + 1]
                    if n4[0] % 2 == 0:
                        nc.scalar.activation(gy2[xt][:xw, :], ps[:xw, :],
                                             mybir.ActivationFunctionType.Copy, scale=sc)
                    else:
                        nc.vector.tensor_scalar(gy2[xt][:xw, :], ps[:xw, :], sc, None, TT.mult)
                    n4[0] += 1
                for hh in range(2):
                    ow = outsp.tile([128, 4 * XS], bf, tag=f"ow{hh}", name=f"ow{hh}")
                    for tp in range(2):
                        ps = psp.tile([128, HALF], f32, tag="ps", name="ps")
                        for half in range(2):
                            t = 4 * hh + 2 * tp + half
                            conv_group(
                                ps[:, half * XS:(half + 1) * XS], sub_kx4,
                                lambda k, rows: gy2[k][:rows, t * 128:(t + 1) * 128])
                        dst = ow[:, 2 * tp * XS:(2 * tp + 2) * XS]
                        if n4[0] % 2 == 0:
                            nc.scalar.copy(dst, ps[:, :])
                        else:
                            nc.vector.tensor_copy(dst, ps[:, :])
                        n4[0] += 1
                    nc.sync.dma_start(
                        out_ext[f, hh * HALF:(hh + 1) * HALF, :].rearrange(
                            "(t p) x -> p t x", p=128),
                        ow[:, :])

    nc.compile()
    return nc, bands_np


_CACHE = {}


def _get_graph():
    if "nc" not in _CACHE:
        _CACHE["nc"], _CACHE["bands"] = build_graph()
    return _CACHE["nc"], _CACHE["bands"]


def host_prep(y, v):
    m = y[:4]
    s = y[4:5]
    v_lr = v[:, ::-1, :].copy()
    v_lr[0] *= -1.0
    vs = 0.5 * (v + v_lr)
    f = np.concatenate([m, s, vs], axis=0).astype(F32)      # [7, Y, X]
    fp = np.pad(f, ((0, 0), (0, 0), (H, H)), mode='edge')
    slabs, masks = [], []
    for c in range(NCORES):
        x0 = c * XS
        slabs.append(np.ascontiguousarray(fp[:, :, x0:x0 + W_IN]).astype(BF16))
        g = x0 + np.arange(W_ALG) - RAD
        mk = ((g >= AP_CUT) & (g < X - AP_CUT)).astype(F32)
        mk_t = np.zeros((128, 3), dtype=F32)
        for xt, (a, w) in enumerate(XT_ALG):
            mk_t[:w, xt] = mk[a:a + w]
        masks.append(mk_t)
    return slabs, masks


def kernel(y, v):
    y = np.asarray(y, dtype=F32)
    v = np.asarray(v, dtype=F32)
    nc, bands_np = _get_graph()
    slabs, masks = host_prep(y, v)
    in_maps = [
        {"x": slabs[c], "bands": bands_np, "mask": masks[c]}
        for c in range(NCORES)
    ]
    res = run_bass_kernel_spmd(nc, in_maps, core_ids=list(range(NCORES)))
    out = np.concatenate([res.results[c]["out"] for c in range(NCORES)], axis=2)
    return out.astype(F32)



# revision 3
# speedup vs baseline: 1.0259x; 1.0259x over previous
"""Distributed Trainium2 Bass kernel for nn_ClosedFlyLoop.

Strategy (8 NeuronCores, shard X into 8 blocks of 256):
 - host: symmetrize v, split y into (m[4], s), pad X edge-replicate by H=25,
   cut per-core overlapping slabs [7, 1024, 306]; ALSO precompute the
   input-only algebra fields hA = c1 + c3*trm, hB = c2, C = c4*trm,
   u1 = m01+m10, u2 = m11-m00 as [5, 1024, 280] slabs (kills the on-device
   ACT affines and Pool combinations); convert to bf16.
 - device, per core (no collectives needed):
     stage 1a: conv along y (circular) as banded matmuls that also transpose
               layout [y,x] -> [x,y']  (lhsT = data chunk, rhs = Toeplitz band)
     stage 1b: conv along x as banded matmuls transposing back [x,y]->[y,x'],
               producing the 14 gradient fields (sign/scale folds baked into
               the band matrices)
     algebra:  pointwise ClosedFlyLoop RHS on VectorE/ScalarE/GpSimd (bf16)
     stage 4a/4b: final Gaussian smooth of the 5 masked pre-fields (same two
               banded-matmul tricks), mask folded into the 4a PSUM evac as a
               per-partition scalar multiply; the C*gamma reaction term rides
               an extra 4a accumulation stream (linear => free adds in PSUM)
 - host: concatenate per-core [5, 1024, 256] f32 outputs along X.
"""
import numpy as np
import ml_dtypes

import concourse.bass as bass
import concourse.bacc as bacc
import concourse.mybir as mybir
from concourse import tile
from concourse.bass_utils import run_bass_kernel_spmd

BF16 = ml_dtypes.bfloat16
F32 = np.float32

Y, X = 1024, 2048
NCORES = 8
XS = X // NCORES            # 256
RAD = 12                    # gauss radius: int(4.0*3.0+0.5)
H = 2 * RAD + 1             # 25
W_IN = XS + 2 * H           # 306
W_ALG = XS + 2 * RAD        # 280
OFF1B = H - RAD             # 13: slab-coord offset of alg window
AP_CUT = 15
YT = Y // 128               # 8 y tiles
XT_IN = [(0, 128), (128, 128), (256, W_IN - 256)]       # x tiles of slab (128,128,50)
XT_ALG = [(0, 128), (128, 128), (256, W_ALG - 256)]     # x tiles of alg width (128,128,24)
HALF = 512                  # y' half width for stage a psum


def _gauss():
    r = RAD
    x = np.arange(-r, r + 1, dtype=np.float64)
    k = np.exp(-0.5 * (x / 3.0) ** 2)
    k = (k / k.sum()).astype(np.float64)
    dk = np.convolve(k, [-0.5, 0.0, 0.5])
    return k.astype(F32), dk.astype(F32)


KERN, DKERN = _gauss()      # 25 taps (rad 12), 27 taps (rad 13)


# ---------------- band submatrix machinery (host) ----------------
class BandPack:
    """Dedup + pack all band submatrices into one [128, K] bf16 constant."""

    def __init__(self):
        self.blocks = {}
        self.cols = []
        self.total = 0

    def add(self, sub):
        sub16 = np.ascontiguousarray(sub.astype(BF16))
        key = (sub16.shape, sub16.tobytes())
        if key not in self.blocks:
            pad = np.zeros((128, sub16.shape[1]), dtype=BF16)
            pad[: sub16.shape[0]] = sub16
            self.blocks[key] = self.total
            self.cols.append(pad)
            self.total += sub16.shape[1]
        return self.blocks[key]

    def packed(self):
        return np.concatenate(self.cols, axis=1)


def band_subs_y(pack, ker, scale):
    """Circular conv along y (1024). Returns per half h: list of
    (chunk k, col a, col b, packed offset, rows) for rhs = B[krows, h*512+a : h*512+b]."""
    r = ker.shape[0] // 2
    B = np.zeros((Y, Y), dtype=F32)
    for j in range(Y):
        for t in range(-r, r + 1):
            B[(j + t) % Y, j] = ker[r + t] * scale
    out = []
    for h in range(2):
        subs = []
        for k in range(YT):
            sub = B[k * 128:(k + 1) * 128, h * HALF:(h + 1) * HALF]
            cols = np.flatnonzero(np.any(sub != 0.0, axis=0))
            if cols.size == 0:
                continue
            a, b = int(cols[0]), int(cols[-1] + 1)
            assert b - a == cols.size
            off = pack.add(sub[:, a:b])
            subs.append((k, a, b, off, 128))
        out.append(subs)
    return out


def band_subs_x(pack, ker, scale, n_in, n_out, off_in):
    """conv along x: out[j] = sum_t kc[t] in[j + off_in + t].
    Returns list of (chunk k, col a, col b, packed offset, rows)."""
    r = ker.shape[0] // 2
    B = np.zeros((n_in, n_out), dtype=F32)
    for j in range(n_out):
        for t in range(-r, r + 1):
            i = j + off_in + t
            if 0 <= i < n_in:
                B[i, j] = ker[r + t] * scale
    subs = []
    nchunks = (n_in + 127) // 128
    for k in range(nchunks):
        rows = min(128, n_in - k * 128)
        sub = B[k * 128:k * 128 + rows, :]
        cols = np.flatnonzero(np.any(sub != 0.0, axis=0))
        if cols.size == 0:
            continue
        a, b = int(cols[0]), int(cols[-1] + 1)
        assert b - a == cols.size
        off = pack.add(sub[:, a:b])
        subs.append((k, a, b, off, rows))
    return subs


# channel order in slab: m00 m01 m10 m11 s v0 v1
# stage-1a D-variant scale per channel (folds signs), stage-1b DK scale per channel
CH_DY_SCALE = [-1.0, -1.0, -1.0, -1.0, -1.0, 1.0, 0.5]   # conv_y(DK) scale
CH_DX_SCALE = [-1.0, -1.0, -1.0, -1.0, -1.0, -0.5, 1.0]  # conv_x(DK) scale
# halg channel order: hA hB C u1 u2
HA, HB, HC, HU1, HU2 = range(5)


def build_graph():
    pack = BandPack()
    sub_ky = band_subs_y(pack, KERN, 1.0)                 # shared smooth-y (also stage 4a)
    # dy via shifted-view matmuls: fold +-0.5*CH_DY_SCALE into the 1b Kx band
    sub_kxd = {}
    for sc in sorted(set(0.5 * s for s in CH_DY_SCALE) | set(-0.5 * s for s in CH_DY_SCALE)):
        sub_kxd[sc] = band_subs_x(pack, KERN, sc, W_IN, W_ALG, OFF1B)
    sub_dkx = {}
    for sc in sorted(set(CH_DX_SCALE)):
        sub_dkx[sc] = band_subs_x(pack, DKERN, sc, W_IN, W_ALG, OFF1B)
    sub_kx4 = band_subs_x(pack, KERN, 1.0, W_ALG, XS, RAD)       # stage 4b
    bands_np = pack.packed()
    KTOT = bands_np.shape[1]

    nc = bacc.Bacc()
    x_ext = nc.declare_dram_parameter("x", [7, Y, W_IN], mybir.dt.bfloat16, isOutput=False)
    halg_ext = nc.declare_dram_parameter("halg", [5, Y, W_ALG], mybir.dt.bfloat16, isOutput=False)
    bands_ext = nc.declare_dram_parameter("bands", [128, KTOT], mybir.dt.bfloat16, isOutput=False)
    mask_ext = nc.declare_dram_parameter("mask", [128, 3], mybir.dt.float32, isOutput=False)
    out_ext = nc.declare_dram_parameter("out", [5, Y, XS], mybir.dt.bfloat16, isOutput=True)

    bf = mybir.dt.bfloat16
    f32 = mybir.dt.float32
    TT = mybir.AluOpType
    evac_ctr = [0]

    with tile.TileContext(nc) as tc:
        with (
            tc.tile_pool(name="const", bufs=1) as constp,
            tc.tile_pool(name="slab", bufs=1) as slabp,
            tc.tile_pool(name="gyt", bufs=1) as gytp,
            tc.tile_pool(name="alg", bufs=3) as algp,
            tc.tile_pool(name="pre", bufs=1) as prep,
            tc.tile_pool(name="gyt2", bufs=3) as gyt2p,
            tc.tile_pool(name="outs", bufs=2) as outsp,
            tc.tile_pool(name="ps", bufs=4, space=bass.MemorySpace.PSUM) as psp,
        ):
            bands = constp.tile([128, KTOT], bf, tag="bands", name="bands")
            nc.sync.dma_start(bands[:, :], bands_ext[:, :])
            # PE p-state warm-up: ~3us of dummy matmuls on the bands constant
            # while the slab DMAs stream, so the real 1a convs start at full
            # clock. Result is never read.
            wps = psp.tile([128, HALF], f32, tag="ps", name="ps")
            nc.tensor.matmul(wps[:, :], bands[:128, 0:128], bands[:128, 0:HALF],
                             start=True, stop=True)
            maskt = constp.tile([128, 3], f32, tag="mask", name="mask")
            nc.sync.dma_start(maskt[:, :], mask_ext[:, :])

            # persistent slab: one wide tile per channel [128, YT*W_IN]; one DMA
            # per channel (128 descriptors of 8*612B rows vs 8 DMAs of 625ns
            # HWDGE overhead each).
            slabw = [slabp.tile([128, YT * W_IN], bf, tag=f"slabw{c}", name=f"slabw{c}")
                     for c in range(7)]
            for c in range(7):
                nc.sync.dma_start(
                    slabw[c][:, :],
                    x_ext[c].rearrange("(t p) x -> p t x", p=128))
            slab = [[slabw[c][:, t * W_IN:(t + 1) * W_IN] for t in range(YT)]
                    for c in range(7)]
            # host-precomputed algebra fields [128, YT*W_ALG] per channel
            halgw = [slabp.tile([128, YT * W_ALG], bf, tag=f"halgw{c}", name=f"halgw{c}")
                     for c in range(5)]
            for c in range(5):
                nc.sync.dma_start(
                    halgw[c][:, :],
                    halg_ext[c].rearrange("(t p) x -> p t x", p=128))

            def evac(dst_ap, src_ap, scale_ap=None):
                # engine deterministic per psum slot (bufs=4 rotation) so each
                # slot is always released by the same engine -> 1 WAR wait.
                slot = evac_ctr[0] % 4
                evac_ctr[0] += 1
                if slot < 2:
                    if scale_ap is None:
                        nc.scalar.copy(dst_ap, src_ap)
                    else:
                        nc.scalar.activation(dst_ap, src_ap,
                                             mybir.ActivationFunctionType.Copy,
                                             scale=scale_ap)
                else:
                    if scale_ap is None:
                        nc.vector.tensor_copy(dst_ap, src_ap)
                    else:
                        nc.vector.tensor_scalar(dst_ap, src_ap, scale_ap, None, TT.mult)

            def conv_group(psum_ap, subs, lhsT_fn):
                n = len(subs)
                for i, (k, a, b, off, rows) in enumerate(subs):
                    nc.tensor.matmul(
                        psum_ap[:, a:b],
                        lhsT_fn(k, rows),
                        bands[:rows, off:off + b - a],
                        start=(i == 0),
                        stop=(i == n - 1),
                    )

            # ---------------- stage 1a: conv_y (Ky only) for all channels ----------------
            # dy rides a raw circular 3-tap diff of gyk along the free y dim;
            # the 0.5 and per-channel signs are folded into the 1b Kx bands.
            # gyk padded: col p holds y = p-1, cols 0/1025 are the circular
            # wrap copies. The dy 3-tap diff is NOT materialized: the 1b
            # dy-path runs two column-shifted matmul groups with +-0.5-scaled
            # bands instead (the diff commutes with the x-conv).
            gyk_all = []
            for c in range(7):
                gyk = [gytp.tile([128, 1026], bf, tag=f"gyk{c}_{xt}", name=f"gyk{c}_{xt}") for xt in range(3)]
                gyk_all.append(gyk)
                for xt, (x0, xw) in enumerate(XT_IN):
                    for h in range(2):
                        ps = psp.tile([128, HALF], f32, tag="ps", name="ps")
                        conv_group(
                            ps[:xw, :], sub_ky[h],
                            lambda k, rows: slab[c][k][:, x0:x0 + xw])
                        evac(gyk[xt][:xw, 1 + h * HALF:1 + (h + 1) * HALF], ps[:xw, :])
                    g = gyk[xt]
                    nc.gpsimd.tensor_copy(g[:xw, 0:1], g[:xw, 1024:1025])
                    nc.gpsimd.tensor_copy(g[:xw, 1025:1026], g[:xw, 1:2])

            # ------- stage 1b + algebra, y-tile-PAIR major: the pointwise
            # algebra runs on [128, 2*280] two-tile batches, halving the
            # per-op fixed cost and the instruction count. PSUM pairs and
            # their evacuations stay per-tile.
            pre = [[prep.tile([128, 2 * W_ALG], bf, tag=f"pre{f}_{tp}", name=f"pre{f}_{tp}")
                    for tp in range(YT // 2)] for f in range(5)]
            dxsubs = {c: sub_dkx[CH_DX_SCALE[c]] for c in range(7)}

            def _emit(ps_view, t, specs):
                first = True
                for gi, (gt, subs, coff) in enumerate(specs):
                    n = len(subs)
                    for i, (k, a, b, off, rows) in enumerate(subs):
                        nc.tensor.matmul(
                            ps_view[:, a:b],
                            gt[k][:rows, t * 128 + coff:t * 128 + coff + 128],
                            bands[:rows, off:off + b - a],
                            start=first,
                            stop=(gi == len(specs) - 1 and i == n - 1),
                        )
                        first = False

            def dy_specs(c):
                sc = 0.5 * CH_DY_SCALE[c]
                return [(gyk_all[c], sub_kxd[sc], 2), (gyk_all[c], sub_kxd[-sc], 0)]

            def grad_pair(t, specsA, specsB):
                """Two conv groups into one 2-bank psum tile (separate zero
                regions at f32 cols 0 and 512) -> one strided evac possible."""
                ps = psp.tile([128, 2 * HALF], f32, tag="pg", name="pg", bufs=2)
                _emit(ps[:, 0:HALF], t, specsA)
                _emit(ps[:, HALF:2 * HALF], t, specsB)
                return ps

            for tp in range(YT // 2):
                ta, tb = 2 * tp, 2 * tp + 1

                def sv2(c):
                    return slabw[c].rearrange("p (t x) -> p t x", x=W_IN)[
                        :, ta:tb + 1, OFF1B:OFF1B + W_ALG]

                def hv2(c):
                    return halgw[c].rearrange("p (t x) -> p t x", x=W_ALG)[
                        :, ta:tb + 1, :]

                def tmp(tag):
                    return algp.tile([128, 2 * W_ALG], bf, tag=tag, name=tag)

                def v3(t_):
                    return t_.rearrange("p (t x) -> p t x", x=W_ALG)

                # w and trE pairs for both tiles; per-tile evac/psum-read into
                # paired tiles, then batched combinations
                wf2 = tmp("wf2"); t1 = tmp("t1")
                for ti, t in enumerate((ta, tb)):
                    ps_wt = grad_pair(
                        t,
                        [(gyk_all[5], dxsubs[5], 1)] + dy_specs(6),
                        dy_specs(5) + [(gyk_all[6], dxsubs[6], 1)])
                    nc.scalar.copy(wf2[:, ti * W_ALG:(ti + 1) * W_ALG], ps_wt[:, :W_ALG])
                    nc.vector.tensor_tensor(
                        t1[:, ti * W_ALG:(ti + 1) * W_ALG],
                        ps_wt[:, HALF:HALF + W_ALG],
                        hv2(HB)[:, ti, :],
                        TT.mult)
                wu1 = tmp("wu1"); wu2 = tmp("wu2"); Ac = tmp("Ac")
                nc.vector.tensor_tensor(v3(wu1), v3(wf2), hv2(HU1), TT.mult)
                nc.gpsimd.tensor_tensor(v3(wu2), v3(wf2), hv2(HU2), TT.mult)
                nc.vector.tensor_tensor(v3(Ac), v3(t1), hv2(HA), TT.add)

                # sdot: per-tile pair evacs into gs2, batched products
                gs2 = algp.tile([128, 4 * W_ALG], bf, tag="gs2", name="gs2")
                gs2v = gs2.rearrange("p (t b x) -> p t b x", b=2, x=W_ALG)
                for ti, t in enumerate((ta, tb)):
                    ps_s = grad_pair(t, dy_specs(4),
                                     [(gyk_all[4], dxsubs[4], 1)])
                    srcv = ps_s.rearrange("p (b x) -> p b x", b=2)[:, :, 0:W_ALG]
                    nc.scalar.copy(gs2v[:, ti, :, :], srcv)
                sd1 = tmp("q1"); sd2 = tmp("q2")
                nc.vector.tensor_tensor(v3(sd1), gs2v[:, :, 0, :], sv2(5), TT.mult)
                nc.vector.tensor_tensor(v3(sd2), gs2v[:, :, 1, :], sv2(6), TT.mult)
                nc.vector.tensor_tensor(pre[4][tp][:, :], sd1[:, :], sd2[:, :], TT.add)

                for ch in range(4):
                    gab2 = algp.tile([128, 4 * W_ALG], bf, tag="gab2", name="gab2", bufs=4)
                    gab2v = gab2.rearrange("p (t b x) -> p t b x", b=2, x=W_ALG)
                    for ti, t in enumerate((ta, tb)):
                        ps_ab = grad_pair(t, dy_specs(ch),
                                          [(gyk_all[ch], dxsubs[ch], 1)])
                        srcv = ps_ab.rearrange("p (b x) -> p b x", b=2)[:, :, 0:W_ALG]
                        nc.scalar.copy(gab2v[:, ti, :, :], srcv)
                    q1 = tmp("q1"); q2 = tmp("q2"); r = tmp("r")
                    nc.vector.tensor_tensor(v3(q1), gab2v[:, :, 0, :], sv2(5), TT.mult)
                    nc.vector.tensor_tensor(v3(q2), gab2v[:, :, 1, :], sv2(6), TT.mult)
                    nc.gpsimd.tensor_tensor(v3(r), v3(Ac), sv2(ch), TT.mult)
                    nc.vector.tensor_tensor(q1[:, :], q1[:, :], q2[:, :], TT.add)
                    nc.vector.tensor_tensor(q1[:, :], q1[:, :], r[:, :], TT.add)
                    p = pre[ch][tp]
                    if ch == 0 or ch == 3:
                        nc.vector.tensor_tensor(
                            p[:, :], q1[:, :], wu1[:, :],
                            TT.subtract if ch == 0 else TT.add)
                    else:
                        nc.vector.tensor_tensor(p[:, :], q1[:, :], wu2[:, :], TT.subtract)

            # ---------------- stage 4: final smooth of 5 fields ----------------
            # 4a: 2-bank psum tiles from the now-idle pg pool, one fused
            # mask-scale evac per (f, xt) alternating ACT/DVE. Field 0 gets an
            # extra accumulation stream for the host-computed C*gamma term.
            # 4b: two y-tiles packed per single psum bank, one evac per pair.
            n4 = [0]
            for f in range(5):
                gy2 = [gyt2p.tile([128, 1024], bf, tag=f"gy2{xt}", name=f"gy2{xt}") for xt in range(3)]
                for xt, (x0, xw) in enumerate(XT_ALG):
                    ps = psp.tile([128, 2 * HALF], f32, tag="pg", name="pg", bufs=2)
                    for h in range(2):
                        # field 0 accumulates a second stream: smooth_y(C), the
                        # host-computed C*gamma_dv reaction term (linear => free
                        # adds in PSUM). Streams share one accumulation group.
                        srcs = [lambda k: pre[f][k // 2][
                            :, (k % 2) * W_ALG + x0:(k % 2) * W_ALG + x0 + xw]]
                        if f == 0:
                            srcs.append(lambda k: halgw[HC][
                                :, k * W_ALG + x0:k * W_ALG + x0 + xw])
                        n_mm = len(srcs) * len(sub_ky[h])
                        j = 0
                        for src in srcs:
                            for (k, a, b, off, rows) in sub_ky[h]:
                                nc.tensor.matmul(
                                    ps[:xw, h * HALF + a:h * HALF + b],
                                    src(k),
                                    bands[:rows, off:off + b - a],
                                    start=(j == 0),
                                    stop=(j == n_mm - 1),
                                )
                                j += 1
                    sc = maskt[:xw, xt:xt + 1]
                    if n4[0] % 2 == 0:
                        nc.scalar.activation(gy2[xt][:xw, :], ps[:xw, :],
                                             mybir.ActivationFunctionType.Copy, scale=sc)
                    else:
                        nc.vector.tensor_scalar(gy2[xt][:xw, :], ps[:xw, :], sc, None, TT.mult)
                    n4[0] += 1
                for hh in range(2):
                    ow = outsp.tile([128, 4 * XS], bf, tag=f"ow{hh}", name=f"ow{hh}")
                    for tp in range(2):
                        ps = psp.tile([128, HALF], f32, tag="ps", name="ps")
                        for half in range(2):
                            t = 4 * hh + 2 * tp + half
                            conv_group(
                                ps[:, half * XS:(half + 1) * XS], sub_kx4,
                                lambda k, rows: gy2[k][:rows, t * 128:(t + 1) * 128])
                        dst = ow[:, 2 * tp * XS:(2 * tp + 2) * XS]
                        if n4[0] % 2 == 0:
                            nc.scalar.copy(dst, ps[:, :])
                        else:
                            nc.vector.tensor_copy(dst, ps[:, :])
                        n4[0] += 1
                    nc.sync.dma_start(
                        out_ext[f, hh * HALF:(hh + 1) * HALF, :].rearrange(
                            "(t p) x -> p t x", p=128),
                        ow[:, :])

    nc.compile()
    return nc, bands_np


_CACHE = {}


def _get_graph():
    if "nc" not in _CACHE:
        _CACHE["nc"], _CACHE["bands"] = build_graph()
    return _CACHE["nc"], _CACHE["bands"]


def host_prep(y, v):
    m = y[:4]
    s = y[4:5]
    v_lr = v[:, ::-1, :].copy()
    v_lr[0] *= -1.0
    vs = 0.5 * (v + v_lr)
    f = np.concatenate([m, s, vs], axis=0).astype(F32)      # [7, Y, X]
    fp = np.pad(f, ((0, 0), (0, 0), (H, H)), mode='edge')
    # input-only algebra fields on the padded grid (pointwise, so pad/compute
    # order is irrelevant for edge replicate)
    sp = fp[4]
    trm = fp[0] + fp[3]
    halg_full = np.stack([
        (-0.11 + 0.099 * sp) + (0.732 - 0.59 * sp) * trm,   # hA
        0.767 + 0.055 * sp,                                  # hB
        (0.069 - 0.048 * sp) * trm,                          # C
        fp[1] + fp[2],                                       # u1
        fp[3] - fp[0],                                       # u2
    ], axis=0)
    slabs, halgs, masks = [], [], []
    for c in range(NCORES):
        x0 = c * XS
        slabs.append(np.ascontiguousarray(fp[:, :, x0:x0 + W_IN]).astype(BF16))
        a0 = x0 + OFF1B
        halgs.append(np.ascontiguousarray(
            halg_full[:, :, a0:a0 + W_ALG]).astype(BF16))
        g = x0 + np.arange(W_ALG) - RAD
        mk = ((g >= AP_CUT) & (g < X - AP_CUT)).astype(F32)
        mk_t = np.zeros((128, 3), dtype=F32)
        for xt, (a, w) in enumerate(XT_ALG):
            mk_t[:w, xt] = mk[a:a + w]
        masks.append(mk_t)
    return slabs, halgs, masks


def kernel(y, v):
    y = np.asarray(y, dtype=F32)
    v = np.asarray(v, dtype=F32)
    nc, bands_np = _get_graph()
    slabs, halgs, masks = host_prep(y, v)
    in_maps = [
        {"x": slabs[c], "halg": halgs[c], "bands": bands_np, "mask": masks[c]}
        for c in range(NCORES)
    ]
    res = run_bass_kernel_spmd(nc, in_maps, core_ids=list(range(NCORES)))
    out = np.concatenate([res.results[c]["out"] for c in range(NCORES)], axis=2)
    return out.astype(F32)


# revision 98
# speedup vs baseline: 1.2122x; 1.1816x over previous
"""Distributed Trainium2 Bass kernel for nn_ClosedFlyLoop.

Strategy (8 NeuronCores, shard X into 8 blocks of 256):
 - host: symmetrize v, split y into (m[4], s), pad X edge-replicate by H=25,
   cut per-core overlapping slabs [7, 1024, 306]; ALSO precompute the
   input-only algebra fields hA = c1 + c3*trm, hB = c2, C = c4*trm,
   u1 = m01+m10, u2 = m11-m00 as [5, 1024, 280] slabs (kills the on-device
   ACT affines and Pool combinations); convert to bf16.
 - device, per core (no collectives needed):
     stage 1a: conv along y (circular) as banded matmuls that also transpose
               layout [y,x] -> [x,y']  (lhsT = data chunk, rhs = Toeplitz band)
     stage 1b: conv along x as banded matmuls transposing back [x,y]->[y,x'],
               producing the 14 gradient fields (sign/scale folds baked into
               the band matrices)
     algebra:  pointwise ClosedFlyLoop RHS on VectorE/ScalarE/GpSimd (bf16)
     stage 4a/4b: final Gaussian smooth of the 5 masked pre-fields (same two
               banded-matmul tricks), mask folded into the 4a PSUM evac as a
               per-partition scalar multiply; the C*gamma reaction term rides
               an extra 4a accumulation stream (linear => free adds in PSUM)
 - host: concatenate per-core [5, 1024, 256] f32 outputs along X.
"""
import numpy as np
import ml_dtypes

import concourse.bass as bass
import concourse.bacc as bacc
import concourse.mybir as mybir
from concourse import tile
from concourse.bass_utils import run_bass_kernel_spmd

BF16 = ml_dtypes.bfloat16
F32 = np.float32

Y, X = 1024, 2048
NCORES = 8
XS = X // NCORES            # 256
RAD = 12                    # gauss radius: int(4.0*3.0+0.5)
H = 2 * RAD + 1             # 25
W_IN = XS + 2 * H           # 306
W_ALG = XS + 2 * RAD        # 280
OFF1B = H - RAD             # 13: slab-coord offset of alg window
AP_CUT = 15
YT = Y // 128               # 8 y tiles
XT_IN = [(0, 128), (128, 128), (256, W_IN - 256)]       # x tiles of slab (128,128,50)
XT_ALG = [(0, 128), (128, 128), (256, W_ALG - 256)]     # x tiles of alg width (128,128,24)
HALF = 512                  # y' half width for stage a psum


def _gauss():
    r = RAD
    x = np.arange(-r, r + 1, dtype=np.float64)
    k = np.exp(-0.5 * (x / 3.0) ** 2)
    k = (k / k.sum()).astype(np.float64)
    dk = np.convolve(k, [-0.5, 0.0, 0.5])
    return k.astype(F32), dk.astype(F32)


KERN, DKERN = _gauss()      # 25 taps (rad 12), 27 taps (rad 13)


# ---------------- band submatrix machinery (host) ----------------
class BandPack:
    """Dedup + pack all band submatrices into one [128, K] bf16 constant."""

    def __init__(self):
        self.blocks = {}
        self.cols = []
        self.total = 0

    def add(self, sub):
        sub16 = np.ascontiguousarray(sub.astype(BF16))
        key = (sub16.shape, sub16.tobytes())
        if key not in self.blocks:
            pad = np.zeros((128, sub16.shape[1]), dtype=BF16)
            pad[: sub16.shape[0]] = sub16
            self.blocks[key] = self.total
            self.cols.append(pad)
            self.total += sub16.shape[1]
        return self.blocks[key]

    def packed(self):
        return np.concatenate(self.cols, axis=1)


def band_subs_y(pack, ker, scale):
    """Circular conv along y (1024). Returns per half h: list of
    (chunk k, col a, col b, packed offset, rows) for rhs = B[krows, h*512+a : h*512+b]."""
    r = ker.shape[0] // 2
    B = np.zeros((Y, Y), dtype=F32)
    for j in range(Y):
        for t in range(-r, r + 1):
            B[(j + t) % Y, j] = ker[r + t] * scale
    out = []
    for h in range(2):
        subs = []
        for k in range(YT):
            sub = B[k * 128:(k + 1) * 128, h * HALF:(h + 1) * HALF]
            cols = np.flatnonzero(np.any(sub != 0.0, axis=0))
            if cols.size == 0:
                continue
            a, b = int(cols[0]), int(cols[-1] + 1)
            assert b - a == cols.size
            off = pack.add(sub[:, a:b])
            subs.append((k, a, b, off, 128, 0))
        out.append(subs)
    return out


def band_subs_x(pack, ker, scale, n_in, n_out, off_in, base64=False):
    """conv along x: out[j] = sum_t kc[t] in[j + off_in + t].
    Returns list of (chunk k, col a, col b, packed offset, rows, base).
    With base64, the last (partial) chunk's block is packed at partition row
    64 (for gyk tiles where a second channel's leftover strip sits at
    partition offset 64 -- lhsT and rhs must share a base partition)."""
    r = ker.shape[0] // 2
    B = np.zeros((n_in, n_out), dtype=F32)
    for j in range(n_out):
        for t in range(-r, r + 1):
            i = j + off_in + t
            if 0 <= i < n_in:
                B[i, j] = ker[r + t] * scale
    subs = []
    nchunks = (n_in + 127) // 128
    for k in range(nchunks):
        rows = min(128, n_in - k * 128)
        sub = B[k * 128:k * 128 + rows, :]
        cols = np.flatnonzero(np.any(sub != 0.0, axis=0))
        if cols.size == 0:
            continue
        a, b = int(cols[0]), int(cols[-1] + 1)
        assert b - a == cols.size
        base = 0
        if base64 and k == nchunks - 1 and rows <= 64:
            pad = np.zeros((64 + rows, b - a), dtype=F32)
            pad[64:] = sub[:, a:b]
            off = pack.add(pad)
            base = 64
        else:
            off = pack.add(sub[:, a:b])
        subs.append((k, a, b, off, rows, base))
    return subs


# channel order in slab: m00 m01 m10 m11 s v0 v1
# stage-1a D-variant scale per channel (folds signs), stage-1b DK scale per channel
CH_DY_SCALE = [-1.0, -1.0, -1.0, -1.0, -1.0, 1.0, 0.5]   # conv_y(DK) scale
CH_DX_SCALE = [-1.0, -1.0, -1.0, -1.0, -1.0, -0.5, 1.0]  # conv_x(DK) scale
# halg channel order: hA hB C u1 u2
HA, HB, HC, HU1, HU2 = range(5)


def build_graph():
    pack = BandPack()
    sub_ky = band_subs_y(pack, KERN, 1.0)                 # shared smooth-y (also stage 4a)
    KY_COLS = pack.total                                  # prefix needed by stage 1a
    # dy via shifted-view matmuls: fold +-0.5*CH_DY_SCALE into the 1b Kx band
    sub_kxd = {}
    sub_kxd64 = {}
    for sc in sorted(set(0.5 * s for s in CH_DY_SCALE) | set(-0.5 * s for s in CH_DY_SCALE)):
        sub_kxd[sc] = band_subs_x(pack, KERN, sc, W_IN, W_ALG, OFF1B)
        sub_kxd64[sc] = band_subs_x(pack, KERN, sc, W_IN, W_ALG, OFF1B, base64=True)
    sub_dkx = {}
    sub_dkx64 = {}
    for sc in sorted(set(CH_DX_SCALE)):
        sub_dkx[sc] = band_subs_x(pack, DKERN, sc, W_IN, W_ALG, OFF1B)
        sub_dkx64[sc] = band_subs_x(pack, DKERN, sc, W_IN, W_ALG, OFF1B, base64=True)
    sub_kyn = band_subs_y(pack, KERN, -1.0)               # negated: subtractive 4a streams
    sub_kx4 = band_subs_x(pack, KERN, 1.0, W_ALG, XS, RAD)       # stage 4b
    bands_np = pack.packed()
    KTOT = bands_np.shape[1]

    nc = bacc.Bacc()
    x_ext = nc.declare_dram_parameter("x", [7, Y, W_IN], mybir.dt.bfloat16, isOutput=False)
    halg_ext = nc.declare_dram_parameter("halg", [5, Y, W_ALG], mybir.dt.bfloat16, isOutput=False)
    bands_ext = nc.declare_dram_parameter("bands", [128, KTOT], mybir.dt.bfloat16, isOutput=False)
    mask_ext = nc.declare_dram_parameter("mask", [128, 4], mybir.dt.float32, isOutput=False)
    out_ext = nc.declare_dram_parameter("out", [5, Y, XS], mybir.dt.bfloat16, isOutput=True)

    bf = mybir.dt.bfloat16
    f32 = mybir.dt.float32
    TT = mybir.AluOpType
    evac_ctr = [0]

    with tile.TileContext(nc) as tc:
        with (
            tc.tile_pool(name="const", bufs=1) as constp,
            tc.tile_pool(name="slab", bufs=1) as slabp,
            tc.tile_pool(name="gyt", bufs=1) as gytp,
            tc.tile_pool(name="alg", bufs=3) as algp,
            tc.tile_pool(name="pre", bufs=1) as prep,
            tc.tile_pool(name="gyt2", bufs=3) as gyt2p,
            tc.tile_pool(name="outs", bufs=2) as outsp,
            tc.tile_pool(name="ps", bufs=4, space=bass.MemorySpace.PSUM) as psp,
        ):
            bands = constp.tile([128, KTOT], bf, tag="bands", name="bands")
            # split the constant DMA: the 1a (Ky) prefix lands before the
            # first slab so 1a can start ~2.3us earlier; the x-band remainder
            # streams after the s slab (first needed by the w/trE pairs).
            nc.sync.dma_start(bands[:, :KY_COLS], bands_ext[:, :KY_COLS])
            # PE p-state warm-up: ~3us of dummy matmuls on the bands constant
            # while the slab DMAs stream, so the real 1a convs start at full
            # clock. Result is never read.
            wps = psp.tile([128, HALF], f32, tag="ps", name="ps", bufs=2)
            for wi in range(3):
                nc.tensor.matmul(wps[:, :448], bands[:128, 0:128],
                                 bands[:128, 0:448],
                                 start=True, stop=True)
            maskt = constp.tile([128, 4], f32, tag="mask", name="mask")

            # persistent slab: one wide tile per channel [128, YT*W_IN]; one DMA
            # per channel. DMA emission order = consumption order: v0, v1, s
            # (then hB, needed by the early trE*hB products) stream first so
            # their 1a convs AND the w/trE/sdot gradient pairs for ALL tiles
            # run while the m-channel slabs + remaining halg fields still
            # stream in the background.
            slabw = [slabp.tile([128, YT * W_IN], bf, tag=f"slabw{c}", name=f"slabw{c}")
                     for c in range(7)]
            halgw = [slabp.tile([128, YT * W_ALG], bf, tag=f"halgw{c}", name=f"halgw{c}")
                     for c in range(5)]

            def dma_slab(c, eng=None):
                (eng or nc.sync).dma_start(
                    slabw[c][:, :],
                    x_ext[c].rearrange("(t p) x -> p t x", p=128))

            def dma_halg(c, eng=None):
                (eng or nc.sync).dma_start(
                    halgw[c][:, :],
                    halg_ext[c].rearrange("(t p) x -> p t x", p=128))

            for c in (5, 6, 4):
                dma_slab(c)
            nc.sync.dma_start(bands[:, KY_COLS:], bands_ext[:, KY_COLS:])
            nc.sync.dma_start(maskt[:, :], mask_ext[:, :])
            dma_halg(HB)
            for c in (0, 1, 2, 3):
                dma_slab(c)
            for hc in (HA, HU1, HU2, HC):
                dma_halg(hc)
            slab = [[slabw[c][:, t * W_IN:(t + 1) * W_IN] for t in range(YT)]
                    for c in range(7)]

            def evac(dst_ap, src_ap, scale_ap=None):
                # rotate drains across ACT/DVE (GPSIMD cannot access PSUM);
                # engine deterministic per psum slot so each slot is always
                # released by the same engine -> 1 WAR wait.
                slot = evac_ctr[0] % 4
                evac_ctr[0] += 1
                if slot < 2:
                    if scale_ap is None:
                        nc.scalar.copy(dst_ap, src_ap)
                    else:
                        nc.scalar.activation(dst_ap, src_ap,
                                             mybir.ActivationFunctionType.Copy,
                                             scale=scale_ap)
                else:
                    if scale_ap is None:
                        nc.vector.tensor_copy(dst_ap, src_ap)
                    else:
                        nc.vector.tensor_scalar(dst_ap, src_ap, scale_ap, None, TT.mult)

            def conv_group(psum_ap, subs, lhsT_fn):
                n = len(subs)
                for i, (k, a, b, off, rows, base) in enumerate(subs):
                    nc.tensor.matmul(
                        psum_ap[:, a:b],
                        lhsT_fn(k, rows),
                        bands[base:base + rows, off:off + b - a],
                        start=(i == 0),
                        stop=(i == n - 1),
                    )

            # ---------------- stage 1a: conv_y (Ky only) ----------------
            # dy rides a raw circular 3-tap diff of gyk along the free y dim;
            # the 0.5 and per-channel signs are folded into the 1b Kx bands.
            # gyk padded: col p holds y = p-1, cols 0/1025 are the circular
            # wrap copies. The dy 3-tap diff is NOT materialized: the 1b
            # dy-path runs two column-shifted matmul groups with +-0.5-scaled
            # bands instead (the diff commutes with the x-conv).
            gyk_all = [None] * 7

            def _1a_conv(c, x0, xw, ps, poff):
                # both y' halves into one 2-bank psum -> single merged drain
                for h in range(2):
                    conv_group(
                        ps[poff:poff + xw, h * HALF:(h + 1) * HALF], sub_ky[h],
                        lambda k, rows: slab[c][k][:, x0:x0 + xw])

            def _1a_drain(dst, srcp, dve_heavy):
                # weighted 3:1 toward ACT for the v/s channels (they land in
                # the DMA-gated head where ACT has idle capacity); 1:3 toward
                # DVE for the m channels (their drains land where ACT is the
                # mid-phase bottleneck)
                slot = evac_ctr[0] % 4
                evac_ctr[0] += 1
                if slot < (2 if dve_heavy else 3):
                    nc.scalar.copy(dst, srcp)
                else:
                    nc.vector.tensor_copy(dst, srcp)

            def do_1a(c, dve_heavy=False):
                gyk = [gytp.tile([128, 1026], bf, tag=f"gyk{c}_{xt}", name=f"gyk{c}_{xt}") for xt in range(2)]
                gyk_all[c] = gyk + [None]
                for xt, (x0, xw) in enumerate(XT_IN[:2]):
                    ps = psp.tile([128, 2 * HALF], f32, tag="pg", name="pg", bufs=3)
                    _1a_conv(c, x0, xw, ps, 0)
                    _1a_drain(gyk[xt][:xw, 1:1025], ps[:xw, :], dve_heavy)
                    g = gyk[xt]
                    nc.gpsimd.tensor_copy(g[:xw, 0:1], g[:xw, 1024:1025])
                    nc.gpsimd.tensor_copy(g[:xw, 1025:1026], g[:xw, 1:2])

            def do_1a_xt2(cs, dve_heavy=False):
                # the 50-wide leftover x-strips of up to 2 channels share one
                # psum tile at partition offsets 0/50 -> one merged drain
                # (drain cost is free-size only; partition count is free)
                x0, xw = XT_IN[2]
                gt = gytp.tile([128, 1026], bf, tag=f"gyk2p_{cs[0]}", name=f"gyk2p_{cs[0]}")
                ps = psp.tile([128, 2 * HALF], f32, tag="pg", name="pg", bufs=3)
                # matmul psum base partition must be 0/32/64: put the second
                # channel at partition 64 (rows 50-63 hold garbage, never read)
                for i, c in enumerate(cs):
                    _1a_conv(c, x0, xw, ps, i * 64)
                    gyk_all[c][2] = gt[i * 64:i * 64 + xw]
                tot = 64 * (len(cs) - 1) + xw
                _1a_drain(gt[:tot, 1:1025], ps[:tot, :], dve_heavy)
                nc.gpsimd.tensor_copy(gt[:tot, 0:1], gt[:tot, 1024:1025])
                nc.gpsimd.tensor_copy(gt[:tot, 1025:1026], gt[:tot, 1:2])

            do_1a(5)
            do_1a(6)
            do_1a_xt2((5, 6))
            do_1a(4)
            do_1a_xt2((4,))

            # ------- stage 1b + algebra, y-tile-PAIR major: the pointwise
            # algebra runs on [128, 2*280] two-tile batches. PSUM pairs and
            # their evacuations stay per-tile. All gradient pairs + drains of
            # a scope are emitted before the dependent product/add chains so
            # no engine FIFO stalls on a single slow producer.
            pre = [[prep.tile([128, 2 * W_ALG], bf, tag=f"pre{f}_{tp}", name=f"pre{f}_{tp}")
                    for tp in range(YT // 2)] for f in range(4)]
            wu1f = [prep.tile([128, 2 * W_ALG], bf, tag=f"wu1f{tp}", name=f"wu1f{tp}")
                    for tp in range(YT // 2)]
            sd1f = [prep.tile([128, 2 * W_ALG], bf, tag=f"sd1f{tp}", name=f"sd1f{tp}")
                    for tp in range(YT // 2)]
            sd2f = [prep.tile([128, 2 * W_ALG], bf, tag=f"sd2f{tp}", name=f"sd2f{tp}")
                    for tp in range(YT // 2)]
            dxsubs = {c: (sub_dkx64 if c in (6, 1, 3) else sub_dkx)[CH_DX_SCALE[c]]
                      for c in range(7)}

            def _emit(ps_view, t, specs):
                first = True
                for gi, (gt, subs, coff) in enumerate(specs):
                    n = len(subs)
                    for i, (k, a, b, off, rows, base) in enumerate(subs):
                        nc.tensor.matmul(
                            ps_view[:, a:b],
                            gt[k][:rows, t * 128 + coff:t * 128 + coff + 128],
                            bands[base:base + rows, off:off + b - a],
                            start=first,
                            stop=(gi == len(specs) - 1 and i == n - 1),
                        )
                        first = False

            OFF64 = (6, 1, 3)   # second channel of each paired xt2 strip

            def dy_specs(c):
                sc = 0.5 * CH_DY_SCALE[c]
                kxd = sub_kxd64 if c in OFF64 else sub_kxd
                return [(gyk_all[c], kxd[sc], 2), (gyk_all[c], kxd[-sc], 0)]

            def grad_pair(t, specsA, specsB):
                """Two conv groups into one 2-bank psum tile (separate zero
                regions at f32 cols 0 and 512) -> one strided evac possible."""
                ps = psp.tile([128, 2 * HALF], f32, tag="pg", name="pg", bufs=3)
                _emit(ps[:, 0:HALF], t, specsA)
                _emit(ps[:, HALF:2 * HALF], t, specsB)
                return ps

            pair_ctr = [0]

            def pair_evac(dst_ap, src_ap):
                # alternate the 1b pair drains ACT/DVE so neither queue backs
                # up the psum slot rotation (psum WAR is what stalls the PE)
                pair_ctr[0] += 1
                nc.scalar.copy(dst_ap, src_ap)

            def hv2(c, tp):
                return halgw[c].rearrange("p (t x) -> p t x", x=W_ALG)[
                    :, 2 * tp:2 * tp + 2, :]

            # ---------------- stage 4: final smooth of 5 fields ----------------
            # 4a: 2-bank psum tiles, one fused mask-scale evac per (f, xt)
            # alternating ACT/DVE; field 0 accumulates the host-computed
            # C*gamma reaction term as a second stream (free adds in PSUM).
            # 4b: two y-tiles packed per single psum bank, one evac per pair.
            def pre_src(fl):
                return lambda k, x0, xw: fl[k // 2][
                    :, (k % 2) * W_ALG + x0:(k % 2) * W_ALG + x0 + xw]

            def halg_src(c):
                return lambda k, x0, xw: halgw[c][
                    :, k * W_ALG + x0:k * W_ALG + x0 + xw]

            F_STREAMS = [
                [(pre_src(pre[0]), sub_ky), (pre_src(wu1f), sub_kyn),
                 (halg_src(HC), sub_ky)],
                [(pre_src(pre[1]), sub_ky)],
                [(pre_src(pre[2]), sub_ky)],
                [(pre_src(pre[3]), sub_ky), (pre_src(wu1f), sub_ky)],
                [(pre_src(sd1f), sub_ky), (pre_src(sd2f), sub_ky)],
            ]
            n4 = [0]

            def _4a_groups(f, ps, x0, xw, poff):
                for h in range(2):
                    n_mm = sum(len(band[h]) for _, band in F_STREAMS[f])
                    j = 0
                    for src, band in F_STREAMS[f]:
                        for (k, a, b, off, rows, base) in band[h]:
                            nc.tensor.matmul(
                                ps[poff:poff + xw, h * HALF + a:h * HALF + b],
                                src(k, x0, xw),
                                bands[base:base + rows, off:off + b - a],
                                start=(j == 0),
                                stop=(j == n_mm - 1),
                            )
                            j += 1

            def _4a_drain(dst, srcp, sc, force_act=False):
                if force_act or n4[0] % 2 == 0:
                    nc.scalar.activation(dst, srcp,
                                         mybir.ActivationFunctionType.Copy, scale=sc)
                else:
                    nc.vector.tensor_scalar(dst, srcp, sc, None, TT.mult)
                n4[0] += 1

            gy2_of = {}

            def emit_4a01(f, force_act=False):
                gy2 = [gyt2p.tile([128, 1024], bf, tag=f"gy2{xt}", name=f"gy2{xt}")
                       for xt in range(2)]
                for xt, (x0, xw) in enumerate(XT_ALG[:2]):
                    ps = psp.tile([128, 2 * HALF], f32, tag="pg", name="pg", bufs=3)
                    _4a_groups(f, ps, x0, xw, 0)
                    _4a_drain(gy2[xt][:xw, :], ps[:xw, :], maskt[:xw, xt:xt + 1],
                              force_act)
                gy2_of[f] = gy2

            def emit_4a2(fs, force_act=False):
                # the 24-wide xt2 strips of up to two FIELDS share one psum
                # (partition offsets 0/64) -> one merged mask-drain; the mask
                # input's col 3 carries the strip mask at both row ranges
                x0, xw = XT_ALG[2]
                gp = gyt2p.tile([128, 1024], bf, tag="gy2p", name="gy2p")
                ps = psp.tile([128, 2 * HALF], f32, tag="pg", name="pg", bufs=3)
                for i, f in enumerate(fs):
                    _4a_groups(f, ps, x0, xw, i * 64)
                    gy2_of[f].append(gp[i * 64:i * 64 + xw])
                tot = 64 * (len(fs) - 1) + xw
                mc = 3 if len(fs) > 1 else 2
                _4a_drain(gp[:tot, :], ps[:tot, :], maskt[:tot, mc:mc + 1],
                          force_act)

            def emit_4b(f, force_act=False):
                gy2 = gy2_of[f]
                kx4 = sub_kx4
                for hh in range(2):
                    ow = outsp.tile([128, 4 * XS], bf, tag=f"ow{hh}", name=f"ow{hh}")
                    for tp_ in range(2):
                        ps = psp.tile([128, HALF], f32, tag="ps", name="ps", bufs=2)
                        for half in range(2):
                            t = 4 * hh + 2 * tp_ + half
                            conv_group(
                                ps[:, half * XS:(half + 1) * XS], kx4,
                                lambda k, rows: gy2[k][:rows, t * 128:(t + 1) * 128])
                        dst = ow[:, 2 * tp_ * XS:(2 * tp_ + 2) * XS]
                        if force_act or n4[0] % 2 == 0:
                            nc.scalar.copy(dst, ps[:, :])
                        else:
                            nc.vector.tensor_copy(dst, ps[:, :])
                        n4[0] += 1
                    nc.sync.dma_start(
                        out_ext[f, hh * HALF:(hh + 1) * HALF, :].rearrange(
                            "(t p) x -> p t x", p=128),
                        ow[:, :])

            # w/trE and sdot gradient pairs for ALL tiles now -- this PE work
            # overlaps the m-channel slab DMAs. Drains land in per-tp
            # persistent tiles (consumed by the per-tp algebra below).
            wf2s = [algp.tile([128, 2 * W_ALG], bf, tag=f"wf2_{tp}", name=f"wf2_{tp}", bufs=1)
                    for tp in range(YT // 2)]
            t1s = [algp.tile([128, 2 * W_ALG], bf, tag=f"t1_{tp}", name=f"t1_{tp}", bufs=1)
                   for tp in range(YT // 2)]
            gs2s = [algp.tile([128, 4 * W_ALG], bf, tag=f"gs2_{tp}", name=f"gs2_{tp}", bufs=1)
                    for tp in range(YT // 2)]
            for tp in range(YT // 2):
                gs2v = gs2s[tp].rearrange("p (t b x) -> p t b x", b=2, x=W_ALG)
                for ti, t in enumerate((2 * tp, 2 * tp + 1)):
                    ps_wt = grad_pair(
                        t,
                        [(gyk_all[5], dxsubs[5], 1)] + dy_specs(6),
                        dy_specs(5) + [(gyk_all[6], dxsubs[6], 1)])
                    pair_evac(wf2s[tp][:, ti * W_ALG:(ti + 1) * W_ALG],
                              ps_wt[:, :W_ALG])
                    # trE*hB reads the psum bank directly (single consumer)
                    nc.vector.tensor_tensor(
                        t1s[tp][:, ti * W_ALG:(ti + 1) * W_ALG],
                        ps_wt[:, HALF:HALF + W_ALG],
                        hv2(HB, tp)[:, ti, :],
                        TT.mult)
                    ps_s = grad_pair(t, dy_specs(4),
                                     [(gyk_all[4], dxsubs[4], 1)])
                    srcv = ps_s.rearrange("p (b x) -> p b x", b=2)[:, :, 0:W_ALG]
                    pair_evac(gs2v[:, ti, :, :], srcv)

            do_1a(0, dve_heavy=True)
            do_1a(1, dve_heavy=True)
            do_1a_xt2((0, 1), dve_heavy=True)
            do_1a(2, dve_heavy=True)
            do_1a(3, dve_heavy=True)
            do_1a_xt2((2, 3), dve_heavy=True)

            for tp in range(YT // 2):
                ta, tb = 2 * tp, 2 * tp + 1
                last = tp == YT // 2 - 1

                def sv2(c):
                    return slabw[c].rearrange("p (t x) -> p t x", x=W_IN)[
                        :, ta:tb + 1, OFF1B:OFF1B + W_ALG]

                def tmp(tag, bufs=2):
                    return algp.tile([128, 2 * W_ALG], bf, tag=tag, name=tag, bufs=bufs)

                def v3(t_):
                    return t_.rearrange("p (t x) -> p t x", x=W_ALG)

                # --- m-channel gradient psum pairs + drains
                gab2s = []
                for ch in range(4):
                    gab2 = algp.tile([128, 4 * W_ALG], bf, tag="gab2", name="gab2", bufs=4)
                    gab2v = gab2.rearrange("p (t b x) -> p t b x", b=2, x=W_ALG)
                    for ti, t in enumerate((ta, tb)):
                        ps_ab = grad_pair(t, dy_specs(ch),
                                          [(gyk_all[ch], dxsubs[ch], 1)])
                        srcv = ps_ab.rearrange("p (b x) -> p b x", b=2)[:, :, 0:W_ALG]
                        pair_evac(gab2v[:, ti, :, :], srcv)
                    gab2s.append(gab2v)

                # --- shared combinations
                gs2v = gs2s[tp].rearrange("p (t b x) -> p t b x", b=2, x=W_ALG)
                wu2 = tmp("wu2"); Ac = tmp("Ac")
                wu1 = wu1f[tp]
                nc.vector.tensor_tensor(
                    wu1.rearrange("p (t x) -> p t x", x=W_ALG),
                    v3(wf2s[tp]), hv2(HU1, tp), TT.mult)
                nc.vector.tensor_tensor(v3(Ac), v3(t1s[tp]), hv2(HA, tp), TT.add)

                # --- all products (every input already drained above)
                nc.vector.tensor_tensor(
                    sd1f[tp].rearrange("p (t x) -> p t x", x=W_ALG),
                    gs2v[:, :, 0, :], sv2(5), TT.mult)
                nc.vector.tensor_tensor(
                    sd2f[tp].rearrange("p (t x) -> p t x", x=W_ALG),
                    gs2v[:, :, 1, :], sv2(6), TT.mult)
                # field 4's smooth sums sd1+sd2 in PSUM (free adds on the PE)
                # and is emitted right after the sd products, before the
                # q-products; its drains are ACT-only so no DVE drain blocks
                # the queued products (FIFO head-of-line).
                if last:
                    emit_4a01(4, force_act=True)
                    emit_4a2((4,), force_act=True)
                    emit_4b(4, force_act=True)
                qs = []
                for ch in range(4):
                    q1 = tmp("q1", bufs=4); q2 = tmp("q2", bufs=4)
                    r = tmp("r", bufs=4)
                    nc.vector.tensor_tensor(v3(q1), gab2s[ch][:, :, 0, :], sv2(5), TT.mult)
                    nc.vector.tensor_tensor(v3(q2), gab2s[ch][:, :, 1, :], sv2(6), TT.mult)
                    nc.gpsimd.tensor_tensor(v3(r), v3(Ac), sv2(ch), TT.mult)
                    qs.append((q1, q2, r))
                # wu2 is only consumed by the last chain ops: emit it after
                # the r products so it does not delay them in the Pool FIFO
                nc.gpsimd.tensor_tensor(v3(wu2), v3(wf2s[tp]), hv2(HU2, tp), TT.mult)

                # --- add chains (inputs all in flight; DVE/Pool split); at the
                # last tile pair, each field's stage-4 smooth is emitted as
                # soon as that field completes (all its PE work is queued
                # behind the last gradient pairs, so no head-of-line block).
                for ch in range(4):
                    q1, q2, r = qs[ch]
                    nc.vector.tensor_tensor(q1[:, :], q1[:, :], q2[:, :], TT.add)
                    p = pre[ch][tp]
                    if ch == 0 or ch == 3:
                        # +-w*u1 rides these fields' extra 4a streams
                        nc.vector.tensor_tensor(p[:, :], q1[:, :], r[:, :], TT.add)
                    elif ch == 1:
                        nc.vector.tensor_tensor(q1[:, :], q1[:, :], r[:, :], TT.add)
                        nc.vector.tensor_tensor(p[:, :], q1[:, :], wu2[:, :], TT.subtract)
                    else:
                        nc.vector.tensor_tensor(q1[:, :], q1[:, :], r[:, :], TT.add)
                        nc.gpsimd.tensor_tensor(p[:, :], q1[:, :], wu2[:, :], TT.subtract)
                    if last:
                        emit_4a01(ch)
                        emit_4a2((ch,))
                        emit_4b(ch)

    nc.compile()
    return nc, bands_np


_CACHE = {}


def _get_graph():
    if "nc" not in _CACHE:
        _CACHE["nc"], _CACHE["bands"] = build_graph()
    return _CACHE["nc"], _CACHE["bands"]


def host_prep(y, v):
    m = y[:4]
    s = y[4:5]
    v_lr = v[:, ::-1, :].copy()
    v_lr[0] *= -1.0
    vs = 0.5 * (v + v_lr)
    f = np.concatenate([m, s, vs], axis=0).astype(F32)      # [7, Y, X]
    fp = np.pad(f, ((0, 0), (0, 0), (H, H)), mode='edge')
    # input-only algebra fields on the padded grid (pointwise, so pad/compute
    # order is irrelevant for edge replicate)
    sp = fp[4]
    trm = fp[0] + fp[3]
    halg_full = np.stack([
        (-0.11 + 0.099 * sp) + (0.732 - 0.59 * sp) * trm,   # hA
        0.767 + 0.055 * sp,                                  # hB
        (0.069 - 0.048 * sp) * trm,                          # C
        fp[1] + fp[2],                                       # u1
        fp[3] - fp[0],                                       # u2
    ], axis=0)
    slabs, halgs, masks = [], [], []
    for c in range(NCORES):
        x0 = c * XS
        slabs.append(np.ascontiguousarray(fp[:, :, x0:x0 + W_IN]).astype(BF16))
        a0 = x0 + OFF1B
        halgs.append(np.ascontiguousarray(
            halg_full[:, :, a0:a0 + W_ALG]).astype(BF16))
        g = x0 + np.arange(W_ALG) - RAD
        mk = ((g >= AP_CUT) & (g < X - AP_CUT)).astype(F32)
        mk_t = np.zeros((128, 4), dtype=F32)
        for xt, (a, w) in enumerate(XT_ALG):
            mk_t[:w, xt] = mk[a:a + w]
        a2, w2 = XT_ALG[2]
        mk_t[64:64 + w2, 3] = mk[a2:a2 + w2]
        mk_t[:w2, 3] = mk[a2:a2 + w2]
        masks.append(mk_t)
    return slabs, halgs, masks


def kernel(y, v):
    y = np.asarray(y, dtype=F32)
    v = np.asarray(v, dtype=F32)
    nc, bands_np = _get_graph()
    slabs, halgs, masks = host_prep(y, v)
    in_maps = [
        {"x": slabs[c], "halg": halgs[c], "bands": bands_np, "mask": masks[c]}
        for c in range(NCORES)
    ]
    res = run_bass_kernel_spmd(nc, in_maps, core_ids=list(range(NCORES)))
    out = np.concatenate([res.results[c]["out"] for c in range(NCORES)], axis=2)
    return out.astype(F32)
